# revision 40
# baseline (speedup 1.0000x reference)
"""ALiBi positional-embedding bias kernel for 8 TRN2 NeuronCores.

Reference math (B=8, H=8, L=1024, TOKEN_NUM=100):
    out[b,h,i,j] = ( tri[i,j] + slopes[h] * base[b,i,j] ) / 5
with tri = -inf on the strict upper triangle (0 elsewhere) and
    base[i,j] = kv[j] + eq[i,j]*(thc[i,j] + resp[j]) - oxth[j]*delta(i,j)
    kv[j]     = j + s2[j] + oxth[j]
    s2[j]     = (101-d[j]) if 101-d[j] > 50.5 else 0
    ox[j]     = 101-d[j] if resp[j]==1 else d[j];  oxth = ox if ox > 50.5 else 0
    eq[i,j]   = [d[i]==d[j]]
    cnt[i,j]  = #{j' <= j : d[j']==d[i]};  thc = cnt if cnt > 102.4 else 0
Since slopes > 0, folding -inf into base before the per-h scale is exact.

Sharding: data-parallel over batch, one batch row per core; slopes
replicated; each core computes its own [H, L, L] block independently.

Output packing: out[b,h,i,j] = -inf for every j >= 128*(i//128 + 1)
regardless of the input data (strict-upper-triangle mask at row-tile
granularity), so the device only materializes, for each row-tile r,
the first w_r = 128*(r+1) key columns (56.25% of the bytes; the
DMA-bound phase shrinks accordingly). Each row-tile r emits 4 groups
of 2 heads packed contiguously at column base 512*r*(r+1); the host
pastes them into a -inf-prefilled [H, L, L] block. Values in the
ragged diagonal block (j in [r0, r0+127], j > i) are computed -inf on
device via affine_select, so the result is bit-identical to the
reference everywhere.
"""

import numpy as np

import concourse.bacc as bacc
import concourse.mybir as mybir
import concourse.tile as tile
from concourse.bass_utils import run_bass_kernel_spmd

B, H, L = 8, 8, 1024
R = L // 128  # row-tiles
TN1 = 101.0  # TOKEN_NUM + 1
N_CORES = 8
FP32 = mybir.dt.float32
BF16 = mybir.dt.bfloat16
I32 = mybir.dt.int32
NEG_INF = float("-inf")

# h -> engine for the per-head scale ops ("a"=ACT, "v"=DVE)
H_ENGINE = ["a", "a", "a", "a", "a", "a", "v", "v"]
OGROUP = 2  # heads per output DMA group

TOTCOL = H * 128 * (R * (R + 1) // 2)  # packed columns per partition


def _colbase(r):
    return H * 128 * (r * (r + 1) // 2)


_CACHED_NC = None


def build_nc():
    nc = bacc.Bacc("TRN2", target_bir_lowering=False, debug=False,
                   num_devices=N_CORES)

    d_ext = nc.dram_tensor("diff", [1, L], FP32, kind="ExternalInput")
    r_ext = nc.dram_tensor("resp", [1, L], FP32, kind="ExternalInput")
    s_ext = nc.dram_tensor("slopes", [1, H], FP32, kind="ExternalInput")
    out_ext = nc.dram_tensor("out", [128, TOTCOL], FP32, kind="ExternalOutput")

    AL = mybir.AluOpType
    F = L // 128  # 8 columns in the transposed layout

    with tile.TileContext(nc) as tc:
        with (
            tc.tile_pool(name="const", bufs=1) as cpool,
            tc.tile_pool(name="work", bufs=3) as wpool,
            tc.tile_pool(name="outp", bufs=3) as opool,
            tc.tile_pool(name="psum", bufs=2, space="PSUM") as ppool,
            tc.tile_pool(name="dram", bufs=1, space="DRAM") as dpool,
        ):
            # ---- inputs: tiny transposed loads + row broadcasts ----
            dT = cpool.tile([128, F], FP32)
            rT = cpool.tile([128, F], FP32)
            db = cpool.tile([128, L], FP32)
            rb = cpool.tile([128, L], FP32)
            slv = cpool.tile([128, H], FP32)
            nc.sync.dma_start(out=dT[:],
                              in_=d_ext[0].rearrange("(p f) -> p f", p=128))
            nc.sync.dma_start(out=rT[:],
                              in_=r_ext[0].rearrange("(p f) -> p f", p=128))
            nc.scalar.dma_start(out=db[:], in_=d_ext[:].to_broadcast([128, L]))
            nc.sync.dma_start(out=rb[:], in_=r_ext[:].to_broadcast([128, L]))
            nc.scalar.dma_start(out=slv[:], in_=s_ext[:].to_broadcast([128, H]))

            slv5 = cpool.tile([128, H], FP32)  # slopes / 5, per-partition scalars
            # on ACT so the activation table load happens early (hidden)
            nc.scalar.mul(slv5[:], slv[:], 0.2)

            # ---- row math on the [128, F] transposed tiles (~100ns/op) ----
            de2T = cpool.tile([128, F], FP32)
            nc.vector.tensor_scalar(de2T[:], dT[:], -1.0, TN1,
                                    op0=AL.mult, op1=AL.add)
            s2T = cpool.tile([128, F], FP32)
            nc.vector.scalar_tensor_tensor(s2T[:], de2T[:], 50.5, de2T[:],
                                           op0=AL.is_gt, op1=AL.mult)
            rdT = cpool.tile([128, F], FP32)
            nc.vector.tensor_mul(rdT[:], rT[:], dT[:])
            u1T = cpool.tile([128, F], FP32)
            nc.vector.scalar_tensor_tensor(u1T[:], rdT[:], -2.0, dT[:],
                                           op0=AL.mult, op1=AL.add)
            oxT = cpool.tile([128, F], FP32)
            nc.vector.scalar_tensor_tensor(oxT[:], rT[:], TN1, u1T[:],
                                           op0=AL.mult, op1=AL.add)
            oxthT = cpool.tile([128, F], FP32)
            nc.vector.scalar_tensor_tensor(oxthT[:], oxT[:], 50.5, oxT[:],
                                           op0=AL.is_gt, op1=AL.mult)

            # kv = j + s2 + oxth
            jTi = cpool.tile([128, F], I32)
            nc.gpsimd.iota(jTi[:], pattern=[[1, F]], channel_multiplier=F)
            jT = cpool.tile([128, F], FP32)
            nc.vector.tensor_copy(out=jT[:], in_=jTi[:])
            kpT = cpool.tile([128, F], FP32)
            nc.vector.tensor_add(kpT[:], jT[:], s2T[:])
            kvT = cpool.tile([128, F], FP32)
            nc.vector.tensor_add(kvT[:], kpT[:], oxthT[:])
            # exact bf16 hi/lo split (kv is integer <= ~1224)
            kv_hiT = cpool.tile([128, F], BF16)
            nc.vector.tensor_copy(out=kv_hiT[:], in_=kvT[:])
            kv_hiTf = cpool.tile([128, F], FP32)
            nc.vector.tensor_copy(out=kv_hiTf[:], in_=kv_hiT[:])
            kv_loT = cpool.tile([128, F], BF16)
            nc.vector.tensor_sub(kv_loT[:], kvT[:], kv_hiTf[:])

            # bounce kv hi/lo rows + an oxth broadcast through DRAM scratch
            scr = dpool.tile([2, L], BF16)
            scro = dpool.tile([1, L], FP32)
            nc.sync.dma_start(
                out=scr[0].rearrange("(p f) -> p f", p=128), in_=kv_hiT[:])
            nc.sync.dma_start(
                out=scr[1].rearrange("(p f) -> p f", p=128), in_=kv_loT[:])
            nc.sync.dma_start(
                out=scro[0].rearrange("(p f) -> p f", p=128), in_=oxthT[:])
            kv_hi = cpool.tile([1, L], BF16)
            kv_lo = cpool.tile([1, L], BF16)
            oxthb = cpool.tile([128, L], FP32)
            nc.sync.dma_start(out=kv_hi[:], in_=scr[0:1, :])
            nc.sync.dma_start(out=kv_lo[:], in_=scr[1:2, :])
            nc.sync.dma_start(out=oxthb[:],
                              in_=scro[:].to_broadcast([128, L]))

            ones_row = cpool.tile([1, 128], BF16)
            nc.vector.memset(ones_row[:], 1.0)

            # onehot[v,j] = [d[j] == v]; whot = onehot * resp[j]
            # whot/cumhot in halves so row-tiles 0-3 (columns 0-511 only)
            # unblock after the first halves.
            iota_p_i = cpool.tile([128, 1], I32)
            nc.gpsimd.iota(iota_p_i[:], pattern=[[0, 1]], channel_multiplier=1)
            iota_p = cpool.tile([128, 1], FP32)
            nc.vector.tensor_copy(out=iota_p[:], in_=iota_p_i[:])
            onehot = cpool.tile([128, L], BF16)
            nc.vector.tensor_scalar(onehot[:], db[:], iota_p[:], None,
                                    op0=AL.is_equal)
            whot = cpool.tile([128, L], BF16)
            cumhot = cpool.tile([128, L], BF16)
            h1 = slice(0, 512)
            h2 = slice(512, L)
            nc.vector.scalar_tensor_tensor(whot[:, h1], db[:, h1], iota_p[:],
                                           rb[:, h1],
                                           op0=AL.is_equal, op1=AL.mult)
            nc.vector.tensor_tensor_scan(cumhot[:, h1], onehot[:, h1],
                                         onehot[:, h1], 0.0,
                                         op0=AL.add, op1=AL.bypass)
            nc.vector.scalar_tensor_tensor(whot[:, h2], db[:, h2], iota_p[:],
                                           rb[:, h2],
                                           op0=AL.is_equal, op1=AL.mult)
            nc.vector.tensor_tensor_scan(cumhot[:, h2], onehot[:, h2],
                                         onehot[:, h2], cumhot[:, 511:512],
                                         op0=AL.add, op1=AL.bypass)

            # ---- main loop over 8 row-tiles. r=3 first (single-chunk,
            # unblocked by the first scan/whot halves) to get the first
            # DMA out early, then largest-first so the DMA engines
            # saturate early and the final tiles drain quickly ----
            for r in (3, 7, 6, 5, 4, 2, 1, 0):
                r0 = r * 128
                w = 128 * (r + 1)  # finite columns for this row-tile
                oh_r = onehot[:, r0:r0 + 128]  # stationary [128v, 128i]
                base_t = wpool.tile([128, L], FP32, tag="base")
                n_chunks = (w + 511) // 512
                for c in range(n_chunks):
                    c0 = c * 512
                    cw = min(512, w - c0)  # DVE width for this chunk
                    sl_c = slice(c0, c0 + 512)
                    sl_v = slice(c0, c0 + cw)
                    p_eq = ppool.tile([128, 512], FP32, tag="eq")
                    p_cnt = ppool.tile([128, 512], FP32, tag="cnt")
                    p_c = ppool.tile([128, 512], FP32, tag="c")
                    nc.tensor.matmul(p_eq[:], oh_r, onehot[:, sl_c])
                    nc.tensor.matmul(p_cnt[:], oh_r, cumhot[:, sl_c])
                    nc.tensor.matmul(p_c[:], oh_r, whot[:, sl_c],
                                     start=True, stop=False)
                    nc.tensor.matmul(p_c[:], ones_row[:], kv_hi[:, sl_c],
                                     start=False, stop=False)
                    nc.tensor.matmul(p_c[:], ones_row[:], kv_lo[:, sl_c],
                                     start=False, stop=True)
                    # thc = cnt*[cnt>102.4]; s4 = thc*eq; base = s4+(s5+kv)
                    g2 = wpool.tile([128, 512], FP32, tag="g2")
                    nc.vector.tensor_scalar(g2[:, :cw], p_cnt[:, :cw],
                                            L * 0.1, None, op0=AL.is_gt)
                    thc = wpool.tile([128, 512], FP32, tag="thc")
                    nc.vector.tensor_mul(thc[:, :cw], g2[:, :cw],
                                         p_cnt[:, :cw])
                    s4 = wpool.tile([128, 512], FP32, tag="s4")
                    nc.vector.tensor_mul(s4[:, :cw], thc[:, :cw],
                                         p_eq[:, :cw])
                    nc.vector.tensor_add(base_t[:, sl_v], s4[:, :cw],
                                         p_c[:, :cw])

                # diagonal fix: base[p, r0+p] -= oxth[r0+p]
                dsel = wpool.tile([128, 128], FP32, tag="dsel")
                nc.gpsimd.affine_select(
                    dsel[:], oxthb[:, r0:r0 + 128],
                    pattern=[[-1, 128]], compare_op=AL.is_equal, fill=0.0,
                    base=0, channel_multiplier=1,
                )
                nc.vector.tensor_sub(base_t[:, r0:r0 + 128],
                                     base_t[:, r0:r0 + 128], dsel[:])

                # causal mask: -inf where j > r0 + p (keep where r0+p-j >= 0)
                nc.gpsimd.affine_select(
                    base_t[:, :w], base_t[:, :w],
                    pattern=[[-1, w]], compare_op=AL.is_ge, fill=NEG_INF,
                    base=r0, channel_multiplier=1,
                )

                # 8 head planes in groups of OGROUP over the w finite
                # columns, packed contiguously per partition in DRAM
                cb = _colbase(r)
                for q in range(H // OGROUP):
                    o_t = opool.tile([128, OGROUP, L], FP32, tag=f"o{q}")
                    for hh in range(OGROUP):
                        h = OGROUP * q + hh
                        if H_ENGINE[h] == "a":
                            nc.scalar.activation(
                                o_t[:, hh, :w], base_t[:, :w],
                                mybir.ActivationFunctionType.Copy,
                                bias=0.0, scale=slv5[:, h:h + 1],
                            )
                        else:
                            nc.vector.tensor_scalar_mul(
                                o_t[:, hh, :w], base_t[:, :w],
                                slv5[:, h:h + 1])
                    dma_eng = nc.sync if q % 2 == 0 else nc.scalar
                    g0 = cb + q * OGROUP * w
                    dst = out_ext[:, g0:g0 + OGROUP * w].rearrange(
                        "p (g w) -> p g w", g=OGROUP)
                    dma_eng.dma_start(out=dst, in_=o_t[:, :, :w])

    nc.compile()
    return nc


def unpack_core(dev):
    """[128, TOTCOL] packed device output -> [H, L, L] full block."""
    dev = np.asarray(dev).reshape(128, TOTCOL)
    out = np.full((H, L, L), NEG_INF, dtype=np.float32)
    for r in range(R):
        w = 128 * (r + 1)
        cb = _colbase(r)
        for q in range(H // OGROUP):
            g0 = cb + q * OGROUP * w
            blk = dev[:, g0:g0 + OGROUP * w].reshape(128, OGROUP, w)
            out[OGROUP * q:OGROUP * (q + 1), r * 128:(r + 1) * 128, :w] = (
                blk.transpose(1, 0, 2))
    return out


def kernel(tensor=None, slopes=None, diff=None, response=None):
    global _CACHED_NC
    if _CACHED_NC is None:
        _CACHED_NC = build_nc()
    nc = _CACHED_NC

    slopes = np.asarray(slopes, dtype=np.float32).reshape(1, H)
    diff_f = np.asarray(diff, dtype=np.float32)
    resp_f = np.asarray(response, dtype=np.float32)

    in_maps = [
        {
            "diff": np.ascontiguousarray(diff_f[b:b + 1, :]),
            "resp": np.ascontiguousarray(resp_f[b:b + 1, :]),
            "slopes": slopes,
        }
        for b in range(B)
    ]
    res = run_bass_kernel_spmd(nc, in_maps, core_ids=list(range(N_CORES)))
    out = np.empty((B, H, L, L), dtype=np.float32)
    for b in range(B):
        out[b] = unpack_core(res.results[b]["out"])
    return out


# revision 52
# speedup vs baseline: 1.0807x; 1.0807x over previous
"""ALiBi positional-embedding bias kernel for 8 TRN2 NeuronCores.

Reference math (B=8, H=8, L=1024, TOKEN_NUM=100):
    out[b,h,i,j] = ( tri[i,j] + slopes[h] * base[b,i,j] ) / 5
with tri = -inf on the strict upper triangle (0 elsewhere) and
    base[i,j] = kv[j] + eq[i,j]*(thc[i,j] + resp[j]) - oxth[j]*delta(i,j)
    kv[j]     = j + s2[j] + oxth[j]
    s2[j]     = (101-d[j]) if 101-d[j] > 50.5 else 0
    ox[j]     = 101-d[j] if resp[j]==1 else d[j];  oxth = ox if ox > 50.5 else 0
    eq[i,j]   = [d[i]==d[j]]
    cnt[i,j]  = #{j' <= j : d[j']==d[i]};  thc = cnt if cnt > 102.4 else 0
Since slopes > 0, folding -inf into base before the per-h scale is exact.

Sharding: data-parallel over batch, one batch row per core; slopes
replicated; each core computes its own [H, L, L] block independently.

Output packing: out[b,h,i,j] = -inf for every j >= 128*(i//128 + 1)
regardless of the input data (strict-upper-triangle mask at row-tile
granularity), so the device only materializes, for each row-tile r,
the first w_r = 128*(r+1) key columns (56.25% of the bytes; the
DMA-bound phase shrinks accordingly). Each row-tile r emits 4 groups
of 2 heads packed contiguously at column base 512*r*(r+1); the host
pastes them into a -inf-prefilled [H, L, L] block. Values in the
ragged diagonal block (j in [r0, r0+127], j > i) are computed -inf on
device via affine_select, so the result is bit-identical to the
reference everywhere.
"""

import numpy as np

import concourse.bacc as bacc
import concourse.mybir as mybir
import concourse.tile as tile
from concourse.bass_utils import run_bass_kernel_spmd
from concourse.tile_rust import add_dep_helper

B, H, L = 8, 8, 1024
R = L // 128  # row-tiles
TN1 = 101.0  # TOKEN_NUM + 1
N_CORES = 8
FP32 = mybir.dt.float32
BF16 = mybir.dt.bfloat16
I32 = mybir.dt.int32
NEG_INF = float("-inf")

# h -> engine for the per-head scale ops ("a"=ACT, "v"=DVE)
H_ENGINE = ["a", "a", "a", "a", "v", "a", "v", "v"]
OGROUP = 2  # heads per output DMA group

TOTCOL = H * 128 * (R * (R + 1) // 2)  # packed columns per partition


def _colbase(r):
    return H * 128 * (r * (r + 1) // 2)


_CACHED_NC = None


def build_nc():
    nc = bacc.Bacc("TRN2", target_bir_lowering=False, debug=False,
                   num_devices=N_CORES)

    d_ext = nc.dram_tensor("diff", [1, L], FP32, kind="ExternalInput")
    r_ext = nc.dram_tensor("resp", [1, L], FP32, kind="ExternalInput")
    s_ext = nc.dram_tensor("slopes", [1, H], FP32, kind="ExternalInput")
    out_ext = nc.dram_tensor("out", [128, TOTCOL], FP32, kind="ExternalOutput")

    AL = mybir.AluOpType
    F = L // 128  # 8 columns in the transposed layout

    with tile.TileContext(nc) as tc:
        with (
            tc.tile_pool(name="const", bufs=1) as cpool,
            tc.tile_pool(name="work", bufs=3) as wpool,
            tc.tile_pool(name="outp", bufs=2) as opool,
            tc.tile_pool(name="outs", bufs=4) as ospool,
            tc.tile_pool(name="psum", bufs=2, space="PSUM") as ppool,
            tc.tile_pool(name="dram", bufs=1, space="DRAM") as dpool,
        ):
            # ---- inputs: tiny transposed loads + row broadcasts ----
            dT = cpool.tile([128, F], FP32)
            rT = cpool.tile([128, F], FP32)
            db = cpool.tile([128, L], FP32)
            rb = cpool.tile([128, L], FP32)
            slv = cpool.tile([128, H], FP32)
            nc.sync.dma_start(out=dT[:],
                              in_=d_ext[0].rearrange("(p f) -> p f", p=128))
            nc.scalar.dma_start(out=rT[:],
                               in_=r_ext[0].rearrange("(p f) -> p f", p=128))
            nc.scalar.dma_start(out=db[:], in_=d_ext[:].to_broadcast([128, L]))
            nc.sync.dma_start(out=rb[:], in_=r_ext[:].to_broadcast([128, L]))
            nc.scalar.dma_start(out=slv[:], in_=s_ext[:].to_broadcast([128, H]))

            slv5 = cpool.tile([128, H], FP32)  # slopes / 5, per-partition scalars
            # on ACT so the activation table load happens early (hidden)
            nc.scalar.mul(slv5[:], slv[:], 0.2)

            # ---- row math on the [128, F] transposed tiles (~100ns/op) ----
            de2T = cpool.tile([128, F], FP32)
            nc.vector.tensor_scalar(de2T[:], dT[:], -1.0, TN1,
                                    op0=AL.mult, op1=AL.add)
            s2T = cpool.tile([128, F], FP32)
            nc.vector.scalar_tensor_tensor(s2T[:], de2T[:], 50.5, de2T[:],
                                           op0=AL.is_gt, op1=AL.mult)
            rdT = cpool.tile([128, F], FP32)
            nc.vector.tensor_mul(rdT[:], rT[:], dT[:])
            u1T = cpool.tile([128, F], FP32)
            nc.vector.scalar_tensor_tensor(u1T[:], rdT[:], -2.0, dT[:],
                                           op0=AL.mult, op1=AL.add)
            oxT = cpool.tile([128, F], FP32)
            nc.vector.scalar_tensor_tensor(oxT[:], rT[:], TN1, u1T[:],
                                           op0=AL.mult, op1=AL.add)
            oxthT = cpool.tile([128, F], FP32)
            nc.vector.scalar_tensor_tensor(oxthT[:], oxT[:], 50.5, oxT[:],
                                           op0=AL.is_gt, op1=AL.mult)

            # kv = j + s2 + oxth
            jTi = cpool.tile([128, F], I32)
            nc.gpsimd.iota(jTi[:], pattern=[[1, F]], channel_multiplier=F)
            jT = cpool.tile([128, F], FP32)
            nc.vector.tensor_copy(out=jT[:], in_=jTi[:])
            kpT = cpool.tile([128, F], FP32)
            nc.vector.tensor_add(kpT[:], jT[:], s2T[:])
            kvT = cpool.tile([128, F], FP32)
            nc.vector.tensor_add(kvT[:], kpT[:], oxthT[:])
            # exact bf16 hi/lo split (kv is integer <= ~1224), packed into
            # one [128, 2F] tile: cols 0..F-1 = hi, F..2F-1 = lo
            kv_hl = cpool.tile([128, 2 * F], BF16)
            nc.vector.tensor_copy(out=kv_hl[:, 0:F], in_=kvT[:])
            kv_hiTf = cpool.tile([128, F], FP32)
            nc.vector.tensor_copy(out=kv_hiTf[:], in_=kv_hl[:, 0:F])
            kv_tail = nc.vector.tensor_sub(kv_hl[:, F:2 * F], kvT[:],
                                           kv_hiTf[:])

            # bounce kv hi|lo rows (sync ring) + an oxth broadcast
            # (scalar ring) through DRAM scratch — two parallel 2-hop chains
            scr = dpool.tile([1, 2 * L], BF16)
            scro = dpool.tile([1, L], FP32)
            nc.sync.dma_start(
                out=scr[0, 0:L].rearrange("(p f) -> p f", p=128),
                in_=kv_hl[:, 0:F])
            nc.sync.dma_start(
                out=scr[0, L:2 * L].rearrange("(p f) -> p f", p=128),
                in_=kv_hl[:, F:2 * F])
            nc.scalar.dma_start(
                out=scro[0].rearrange("(p f) -> p f", p=128), in_=oxthT[:])
            kvrow = cpool.tile([1, 2 * L], BF16)
            oxthb = cpool.tile([128, L], FP32)
            nc.sync.dma_start(out=kvrow[:], in_=scr[:])
            nc.scalar.dma_start(out=oxthb[:],
                                in_=scro[:].to_broadcast([128, L]))
            kv_hi = kvrow[:, 0:L]
            kv_lo = kvrow[:, L:2 * L]

            ones_row = cpool.tile([1, 128], BF16)
            nc.vector.memset(ones_row[:], 1.0)

            # onehot[v,j] = [d[j] == v]; whot = onehot * resp[j]
            # whot/cumhot in halves so row-tiles 0-3 (columns 0-511 only)
            # unblock after the first halves.
            iota_p_i = cpool.tile([128, 1], I32)
            nc.gpsimd.iota(iota_p_i[:], pattern=[[0, 1]], channel_multiplier=1)
            iota_p = cpool.tile([128, 1], FP32)
            nc.vector.tensor_copy(out=iota_p[:], in_=iota_p_i[:])
            onehot = cpool.tile([128, L], BF16)
            i_oh = nc.vector.tensor_scalar(onehot[:], db[:], iota_p[:], None,
                                           op0=AL.is_equal)
            whot = cpool.tile([128, L], BF16)
            cumhot = cpool.tile([128, L], BF16)
            h1 = slice(0, 512)
            h2 = slice(512, L)
            i_w1 = nc.vector.scalar_tensor_tensor(whot[:, h1], db[:, h1],
                                                  iota_p[:], rb[:, h1],
                                                  op0=AL.is_equal,
                                                  op1=AL.mult)
            i_s1 = nc.vector.tensor_tensor_scan(cumhot[:, h1], onehot[:, h1],
                                                onehot[:, h1], 0.0,
                                                op0=AL.add, op1=AL.bypass)
            i_w2 = nc.vector.scalar_tensor_tensor(whot[:, h2], db[:, h2],
                                                  iota_p[:], rb[:, h2],
                                                  op0=AL.is_equal,
                                                  op1=AL.mult)
            i_s2 = nc.vector.tensor_tensor_scan(cumhot[:, h2], onehot[:, h2],
                                                onehot[:, h2],
                                                cumhot[:, 511:512],
                                                op0=AL.add, op1=AL.bypass)
            # keep the kv hi/lo tail (gates the scratch round-trip and the
            # p_c matmuls) ahead of the long DVE ops in the schedule
            for big in (i_oh, i_w1, i_s1, i_w2, i_s2):
                add_dep_helper(big.ins, kv_tail.ins, sync=False,
                               reason="prioritize kv round-trip")

            # ---- main loop over 8 row-tiles. r=3 first (single-chunk,
            # unblocked by the first scan/whot halves) to get the first
            # DMA out early, then largest-first so the DMA engines
            # saturate early and the final tiles drain quickly ----
            for r in (3, 7, 6, 5, 4, 2, 1, 0):
                r0 = r * 128
                w = 128 * (r + 1)  # finite columns for this row-tile
                oh_r = onehot[:, r0:r0 + 128]  # stationary [128v, 128i]
                base_t = wpool.tile([128, L], FP32, tag="base")
                n_chunks = (w + 511) // 512
                for c in range(n_chunks):
                    c0 = c * 512
                    cw = min(512, w - c0)  # DVE width for this chunk
                    sl_c = slice(c0, c0 + 512)
                    sl_v = slice(c0, c0 + cw)
                    p_eq = ppool.tile([128, 512], FP32, tag="eq")
                    p_cnt = ppool.tile([128, 512], FP32, tag="cnt")
                    p_c = ppool.tile([128, 512], FP32, tag="c")
                    nc.tensor.matmul(p_eq[:], oh_r, onehot[:, sl_c])
                    nc.tensor.matmul(p_cnt[:], oh_r, cumhot[:, sl_c])
                    nc.tensor.matmul(p_c[:], oh_r, whot[:, sl_c],
                                     start=True, stop=False)
                    nc.tensor.matmul(p_c[:], ones_row[:], kv_hi[:, sl_c],
                                     start=False, stop=False)
                    nc.tensor.matmul(p_c[:], ones_row[:], kv_lo[:, sl_c],
                                     start=False, stop=True)
                    # thc = cnt*[cnt>102.4]; s4 = thc*eq; base = s4+(s5+kv)
                    g2 = wpool.tile([128, 512], FP32, tag="g2")
                    nc.vector.tensor_scalar(g2[:, :cw], p_cnt[:, :cw],
                                            L * 0.1, None, op0=AL.is_gt)
                    thc = wpool.tile([128, 512], FP32, tag="thc")
                    nc.vector.tensor_mul(thc[:, :cw], g2[:, :cw],
                                         p_cnt[:, :cw])
                    s4 = wpool.tile([128, 512], FP32, tag="s4")
                    nc.vector.tensor_mul(s4[:, :cw], thc[:, :cw],
                                         p_eq[:, :cw])
                    nc.vector.tensor_add(base_t[:, sl_v], s4[:, :cw],
                                         p_c[:, :cw])

                # diagonal fix: base[p, r0+p] -= oxth[r0+p]
                dsel = wpool.tile([128, 128], FP32, tag="dsel")
                nc.gpsimd.affine_select(
                    dsel[:], oxthb[:, r0:r0 + 128],
                    pattern=[[-1, 128]], compare_op=AL.is_equal, fill=0.0,
                    base=0, channel_multiplier=1,
                )
                nc.vector.tensor_sub(base_t[:, r0:r0 + 128],
                                     base_t[:, r0:r0 + 128], dsel[:])

                # causal mask: -inf where j > r0 + p (keep where r0+p-j >= 0)
                nc.gpsimd.affine_select(
                    base_t[:, :w], base_t[:, :w],
                    pattern=[[-1, w]], compare_op=AL.is_ge, fill=NEG_INF,
                    base=r0, channel_multiplier=1,
                )

                # 8 head planes in groups of OGROUP over the w finite
                # columns, packed contiguously per partition in DRAM
                cb = _colbase(r)
                for q in range(H // OGROUP):
                    if w <= 512:
                        o_t = ospool.tile([128, OGROUP, 512], FP32,
                                          tag=f"s{q}")
                    else:
                        o_t = opool.tile([128, OGROUP, L], FP32, tag=f"o{q}")
                    for hh in range(OGROUP):
                        h = OGROUP * q + hh
                        if H_ENGINE[h] == "a":
                            nc.scalar.activation(
                                o_t[:, hh, :w], base_t[:, :w],
                                mybir.ActivationFunctionType.Copy,
                                bias=0.0, scale=slv5[:, h:h + 1],
                            )
                        else:
                            nc.vector.tensor_scalar_mul(
                                o_t[:, hh, :w], base_t[:, :w],
                                slv5[:, h:h + 1])
                    dma_eng = nc.sync if q % 2 == 0 else nc.scalar
                    g0 = cb + q * OGROUP * w
                    dst = out_ext[:, g0:g0 + OGROUP * w].rearrange(
                        "p (g w) -> p g w", g=OGROUP)
                    dma_eng.dma_start(out=dst, in_=o_t[:, :, :w])

    nc.compile()
    return nc


def unpack_core(dev):
    """[128, TOTCOL] packed device output -> [H, L, L] full block."""
    dev = np.asarray(dev).reshape(128, TOTCOL)
    out = np.full((H, L, L), NEG_INF, dtype=np.float32)
    for r in range(R):
        w = 128 * (r + 1)
        cb = _colbase(r)
        for q in range(H // OGROUP):
            g0 = cb + q * OGROUP * w
            blk = dev[:, g0:g0 + OGROUP * w].reshape(128, OGROUP, w)
            out[OGROUP * q:OGROUP * (q + 1), r * 128:(r + 1) * 128, :w] = (
                blk.transpose(1, 0, 2))
    return out


def kernel(tensor=None, slopes=None, diff=None, response=None):
    global _CACHED_NC
    if _CACHED_NC is None:
        _CACHED_NC = build_nc()
    nc = _CACHED_NC

    slopes = np.asarray(slopes, dtype=np.float32).reshape(1, H)
    diff_f = np.asarray(diff, dtype=np.float32)
    resp_f = np.asarray(response, dtype=np.float32)

    in_maps = [
        {
            "diff": np.ascontiguousarray(diff_f[b:b + 1, :]),
            "resp": np.ascontiguousarray(resp_f[b:b + 1, :]),
            "slopes": slopes,
        }
        for b in range(B)
    ]
    res = run_bass_kernel_spmd(nc, in_maps, core_ids=list(range(N_CORES)))
    out = np.empty((B, H, L, L), dtype=np.float32)
    for b in range(B):
        out[b] = unpack_core(res.results[b]["out"])
    return out
